# revision 11
# baseline (speedup 1.0000x reference)
"""Trainium2 Bass kernel for 16-head MHA (B=4, S=2048, D=1024), 8 NeuronCores.

Sharding: 4-way data parallel on batch x 2-way tensor parallel on heads.
Core c handles batch c//2, head-group c%2 (8 heads, d_model slice of 512).
Host sums the two partial out-projections per batch and adds bo.

v4 design (from trace analysis of v2 424us / v3 419us):
  - The EXP stream (256 x [128,1024] ACTIVATE, ~285us) is the pacing
    engine; everything else must hide inside it.  v3 showed two
    blockers: DMA *issue* serialization (~650ns/instruction on the
    Sync queue, 80 instructions -> first EXP at 42us) and PE
    saturation (per-chunk attention matmul pitch ~1.0us vs the 1.11us
    EXP window leaves no room for projection fillers).
  - v4 batches each input tensor(-half) into ONE DMA with a 3D access
    pattern (12 issues instead of 80): first EXP ~10us.
  - x, W, V-path and attention-output tensors move to fp8e4m3 with
    DoubleRow matmuls (2 contraction tiles per pass): projections,
    PV and out-projection take half the PE slots, so fillers fit in
    the EXP-window slack.  Weights are scaled x16 on the host (their
    raw magnitude ~0.02 sits at the fp8 subnormal edge); the scale is
    unwound via the EXP scale (/256) and the host output scale (/256).
  - Scores stay bf16 (kh/qh) with the dual-row-group co-start; exp
    reads fp32 PSUM scores and writes fp8 attention weights directly.
  - Attention block = (head-pair hp, q-block of 512).  Score tile
    [128,1024] = {A|B} x 512q (sc: 2 slots = 4 banks), PV accumulators
    [65,512] (pv: 2 slots = 2 banks), projections/out-proj on the
    remaining 2 banks ("fp" pool), interleaved as per-chunk fillers.
  - PV keeps the ones-column trick (M=65: the softmax denominator
    rides the P stream for free) and accumulates chunk PAIRS via
    DoubleRow ([128,2,65] x [128,2,512]).
"""

import sys

if "/opt/trn_rl_repo" not in sys.path:
    sys.path.insert(0, "/opt/trn_rl_repo")

import numpy as np
import ml_dtypes

S = 2048          # sequence length
D = 1024          # d_model
DL = 512          # local d_model slice (8 heads * 64)
H = 8             # local heads
DK = 64           # head dim
NB = 4            # batches
NG = 2            # head groups
KC = S // 128     # 16 k-chunks
SW = 1.0          # no weight scaling needed at bf16
BF16 = ml_dtypes.bfloat16
E4M3 = ml_dtypes.float8_e4m3

_cache = {}


def _build_nc():
    import concourse.bass as bass
    import concourse.mybir as mybir
    import concourse.tile as tile
    from concourse import bacc

    f32 = mybir.dt.float32
    bf = mybir.dt.bfloat16
    f8 = mybir.dt.float8e4
    DR = mybir.MatmulPerfMode.DoubleRow

    nc = bacc.Bacc(None, target_bir_lowering=False)

    xqT = nc.dram_tensor("xqT", [D, S], bf, kind="ExternalInput")
    xkT = nc.dram_tensor("xkT", [D, S], bf, kind="ExternalInput")
    xvT = nc.dram_tensor("xvT", [D, S], bf, kind="ExternalInput")
    wqT = nc.dram_tensor("wqT", [D, DL], bf, kind="ExternalInput")
    wkT = nc.dram_tensor("wkT", [D, DL], bf, kind="ExternalInput")
    wvT = nc.dram_tensor("wvT", [D, DL], bf, kind="ExternalInput")
    woT = nc.dram_tensor("woT", [DL, D], bf, kind="ExternalInput")
    bq2 = nc.dram_tensor("bq2", [128, 4], f32, kind="ExternalInput")
    bk2 = nc.dram_tensor("bk2", [128, 4], f32, kind="ExternalInput")
    yT = nc.dram_tensor("yT", [D, S], bf, kind="ExternalOutput")

    Exp = mybir.ActivationFunctionType.Exp

    with tile.TileContext(nc) as tc:
        with (
            tc.tile_pool(name="consts", bufs=1) as consts,
            tc.tile_pool(name="wpool", bufs=1) as wpool,
            tc.tile_pool(name="xpool", bufs=3) as xpool,
            tc.tile_pool(name="qkpool", bufs=1) as qkpool,
            tc.tile_pool(name="vpool", bufs=1) as vpool,
            tc.tile_pool(name="ppool", bufs=2) as ppool,
            tc.tile_pool(name="dpool", bufs=2) as dpool,
            tc.tile_pool(name="ypool", bufs=2) as ypool,
            tc.tile_pool(name="psum", bufs=1, space="PSUM") as psum,
        ):
            # ---- biases ----
            bq_sb = consts.tile([128, 4], f32)
            nc.sync.dma_start(bq_sb[:], bq2[:])
            bk_sb = consts.tile([128, 4], f32)
            nc.sync.dma_start(bk_sb[:], bk2[:])

            # ---- batched input DMAs: one instruction per tensor.
            # dest [128, 8, 2048]: partition p, d-chunk a, seq s; source
            # row a*128+p -> 3D strided AP, 2KB contiguous lines.
            def load_x(srct, nm, tag):
                t = xpool.tile([128, 8, 2048], bf, tag=tag, bufs=1, name=nm)
                nc.sync.dma_start(
                    t[:], srct.rearrange("(a p) s -> p a s", p=128))
                return t

            def load_w(srct, nm):
                t = wpool.tile([128, 8, 512], bf, name=nm)
                nc.sync.dma_start(
                    t[:], srct.rearrange("(a p) m -> p a m", p=128))
                return t

            wk_all = load_w(wkT, "wk")
            xk_sb = load_x(xkT, "xk", "xk")
            wq_all = load_w(wqT, "wq")
            xq_sb = load_x(xqT, "xq", "xq")
            wv_all = load_w(wvT, "wv")
            xv_sb = load_x(xvT, "xv", "xv_wo")
            # wo rides the xv slot (xv's last reader, Vr(15), is emitted in
            # block (1,0); the first O-projection reads wo in block (3,1))
            wo_all = xpool.tile([128, 8, 2048], bf, tag="xv_wo", bufs=1, name="wo")
            nc.sync.dma_start(
                wo_all[:, 0:4, 0:1024], woT.rearrange("(a p) m -> p a m", p=128))

            # ---- HAM warmup: keep PE busy through the first input-DMA
            # window so the projection matmuls start at 2.4GHz.
            wtile = consts.tile([128, 64], bf, name="warm")
            nc.vector.memset(wtile[:], 0.0)
            wps = psum.tile([128, 64], mybir.dt.float32, tag="fp", bufs=2,
                            name="warmps")
            for i in range(100):
                nc.tensor.matmul(wps[0:64, :], lhsT=wtile[:, 0:64],
                                 rhs=wtile[:], start=True, stop=True)

            # ---- persistent activation tiles ----
            qh_sb = [qkpool.tile([128, S], bf, name=f"qh{i}") for i in range(4)]
            kh_sb = [qkpool.tile([128, S], bf, name=f"kh{i}") for i in range(4)]
            aoall = qkpool.tile([128, 4, S], bf, name="ao")
            # per-head stride padded to 66 so the PV DoubleRow k-tile stride
            # (8*66=528B) is 16B-aligned (ISA requires step%16==0)
            vhall = vpool.tile([128, KC, H, DK + 1], bf, name="vh")

            # ones columns for the PV z-row trick
            nc.vector.memset(vhall[:, :, :, DK:DK + 1], 1.0)

            # ---- one projection output block: features mc*128, seq st*512,
            # fp8 DoubleRow over d-chunk pairs ----
            def emit_proj(w_all, x_sb, o_sb, b_sb, nm, mc, st):
                ps = psum.tile([128, 512], mybir.dt.float32,
                               tag="fp", bufs=2,
                               name=f"ps{nm}{mc}_{st}")
                for j in range(8):
                    nc.tensor.matmul(
                        ps[:],
                        lhsT=w_all[:, j, mc * 128:(mc + 1) * 128],
                        rhs=x_sb[:, j, st * 512:(st + 1) * 512],
                        start=(j == 0),
                        stop=(j == 7),
                    )
                nc.vector.tensor_scalar_add(
                    o_sb[mc][:, st * 512:(st + 1) * 512],
                    ps[:],
                    b_sb[:, mc:mc + 1],
                )

            def K(mc, st):
                emit_proj(wk_all, xk_sb, kh_sb, bk_sb, "k", mc, st)

            def Q(mc, st):
                emit_proj(wq_all, xq_sb, qh_sb, bq_sb, "q", mc, st)

            # ---- V-projection slices (natural [k, head, dk] layout) ----
            def _vproj(c, col0, col1, hlo, hhi):
                ps = psum.tile([128, col1 - col0], mybir.dt.float32,
                               tag="fp", bufs=2, name=f"psv{c}_{hlo}")
                for j in range(8):
                    nc.tensor.matmul(
                        ps[:],
                        lhsT=xv_sb[:, j, c * 128:(c + 1) * 128],
                        rhs=wv_all[:, j, col0:col1],
                        start=(j == 0),
                        stop=(j == 7),
                    )
                nc.vector.tensor_copy(
                    vhall[:, c, hlo:hhi, 0:DK],
                    ps.rearrange("p (h d) -> p h d", h=hhi - hlo),
                )

            def Vs(c):
                _vproj(c, 0, 128, 0, 2)      # heads 0,1 (hp group 0)

            def Vr(c):
                _vproj(c, 128, 512, 2, 8)    # heads 2..7

            # ---- out-projection group: out rows oc*128, seq block st ----
            def O(st, oc, tail=False):
                ps = psum.tile([128, 512], mybir.dt.float32,
                               tag="fp", bufs=2,
                               name=f"pso{oc}_{st}")
                for j in range(4):
                    nc.tensor.matmul(
                        ps[:],
                        lhsT=wo_all[:, j, oc * 128:(oc + 1) * 128],
                        rhs=aoall[:, j, st * 512:(st + 1) * 512],
                        start=(j == 0),
                        stop=(j == 3),
                    )
                yt = ypool.tile([128, 512], bf, tag="yt", bufs=2,
                                name=f"yt{oc}_{st}")
                if tail:
                    nc.scalar.copy(yt[:], ps[:])
                else:
                    nc.vector.tensor_copy(yt[:], ps[:])
                nc.sync.dma_start(
                    yT[oc * 128:(oc + 1) * 128, st * 512:(st + 1) * 512],
                    yt[:],
                )

            # ---- one attention block: head pair hp, q columns qb*512 ----
            def emit_block(hp, qb, fillers=None):
                fillers = fillers or {}
                q0 = qb * 512
                pvA = psum.tile([65, 512], mybir.dt.float32, tag="pv",
                                bufs=2, name=f"pvA{hp}_{qb}")
                pvB = psum.tile([65, 512], mybir.dt.float32, tag="pv",
                                bufs=2, name=f"pvB{hp}_{qb}")
                pts = [None] * KC

                def emit_pv(c):
                    for i, pvt in ((0, pvA), (1, pvB)):
                        nc.tensor.matmul(
                            pvt[:],
                            lhsT=vhall[:, c, 2 * hp + i, :],
                            rhs=pts[c][:, i * 512:(i + 1) * 512],
                            start=(c == 0), stop=(c == KC - 1),
                        )

                for c in range(KC):
                    s = psum.tile([128, 1024], mybir.dt.float32, tag="sc",
                                  bufs=2, name=f"s{hp}_{qb}_{c}")
                    nc.tensor.matmul(
                        s[:, 0:512],
                        lhsT=kh_sb[hp][0:64, c * 128:(c + 1) * 128],
                        rhs=qh_sb[hp][0:64, q0:q0 + 512],
                        start=True, stop=True,
                        tile_position=(0, 0),
                    )
                    nc.tensor.matmul(
                        s[:, 512:1024],
                        lhsT=kh_sb[hp][64:128, c * 128:(c + 1) * 128],
                        rhs=qh_sb[hp][64:128, q0:q0 + 512],
                        start=True, stop=True,
                        tile_position=(64, 0),
                    )
                    pt = ppool.tile([128, 1024], bf, tag="pa",
                                    bufs=3, name=f"p{hp}_{qb}_{c}")
                    nc.scalar.activation(pt[:], s[:], Exp, scale=0.125)
                    pts[c] = pt
                    # software-pipelined PV: consume chunk c-1 (whose exp
                    # retired before exp(c) started) while chunk c exps
                    if c > 0:
                        emit_pv(c - 1)
                    for fn in fillers.get(c, ()):
                        fn()

                emit_pv(KC - 1)

                # normalization straight out of PSUM; only the z row is
                # staged to SBUF (DMA cannot read PSUM) for the
                # partition-0 move that custom-DVE recip/broadcast need.
                for i, pvt in ((0, pvA), (1, pvB)):
                    qsl = slice(q0, q0 + 512)
                    pvs = dpool.tile([65, 512], bf, tag="zs", bufs=2,
                                     name=f"pvs{hp}_{qb}_{i}")
                    nc.vector.tensor_copy(pvs[:], pvt[:])
                    z0b = dpool.tile([1, 512], bf, tag="z0b", bufs=1,
                                     name=f"z0b{hp}_{qb}_{i}")
                    nc.gpsimd.dma_start(z0b[:], pvs[64:65, :])
                    z0 = dpool.tile([1, 512], f32, tag="z0", bufs=1,
                                    name=f"z0{hp}_{qb}_{i}")
                    nc.vector.tensor_copy(z0[:], z0b[:])
                    nc.vector.reciprocal_approx_fast(z0[:], z0[:])
                    bc = dpool.tile([64, 512], f32, tag="bc", bufs=1,
                                    name=f"bc{hp}_{qb}_{i}")
                    nc.gpsimd.partition_broadcast(bc[:], z0[:])
                    # v-bias is folded into the host-side output bias
                    head = 2 * hp + i
                    if i == 0:
                        nc.vector.tensor_mul(aoall[0:64, hp, qsl],
                                             pvs[0:64, :], bc[:])
                    else:
                        stg = dpool.tile([64, 512], f8, tag="stg", bufs=1,
                                         name=f"stg{hp}_{qb}_{i}")
                        nc.vector.tensor_mul(stg[:], pvs[0:64, :], bc[:])
                        nc.gpsimd.dma_start(aoall[64:128, hp, qsl], stg[:])

            # ================= emission schedule =================
            # pre-phase: just enough for block (0,0) to start (~10us).
            K(0, 0)
            Q(0, 0)

            SCHED = {
                (0, 0): {0: [lambda: Vs(0), lambda: Vs(1)],
                         1: [lambda: K(0, 1)],
                         2: [lambda: Vs(2), lambda: Vs(3)],
                         3: [lambda: Vs(4), lambda: Vs(5)],
                         4: [lambda: Vs(6), lambda: Vs(7)],
                         5: [lambda: Vs(8), lambda: Vs(9)],
                         6: [lambda: Vs(10), lambda: Vs(11), lambda: K(0, 2)],
                         7: [lambda: Vs(12), lambda: Vs(13)],
                         8: [lambda: Vs(14), lambda: Vs(15)],
                         10: [lambda: K(0, 3)],
                         13: [lambda: Q(0, 1)]},
                (0, 1): {1: [lambda: Vr(0)], 3: [lambda: Vr(1)],
                         5: [lambda: Vr(2)], 7: [lambda: Vr(3)],
                         9: [lambda: Q(0, 2)], 11: [lambda: Vr(4)]},
                (0, 2): {1: [lambda: Vr(5)], 3: [lambda: Vr(6)],
                         5: [lambda: Vr(7)], 7: [lambda: Vr(8)],
                         9: [lambda: Q(0, 3)], 11: [lambda: Vr(9)]},
                (0, 3): {1: [lambda: Vr(10)], 3: [lambda: Vr(11)],
                         5: [lambda: Vr(12)], 7: [lambda: K(1, 0)],
                         9: [lambda: Q(1, 0)], 11: [lambda: Vr(13)]},
                (1, 0): {1: [lambda: Vr(14)], 2: [lambda: K(1, 1)],
                         4: [lambda: Vr(15)], 6: [lambda: K(1, 2)],
                         9: [lambda: Q(1, 1)], 10: [lambda: K(1, 3)]},
                (1, 1): {2: [lambda: Q(1, 2)], 5: [lambda: K(2, 0)]},
                (1, 2): {2: [lambda: Q(1, 3)], 5: [lambda: K(2, 1)]},
                (1, 3): {2: [lambda: Q(2, 0)], 5: [lambda: K(2, 2)],
                         8: [lambda: K(2, 3)]},
                (2, 0): {2: [lambda: Q(2, 1)], 5: [lambda: K(3, 0)]},
                (2, 1): {2: [lambda: Q(2, 2)], 5: [lambda: K(3, 1)]},
                (2, 2): {2: [lambda: Q(2, 3)], 5: [lambda: K(3, 2)]},
                (2, 3): {2: [lambda: Q(3, 0)], 5: [lambda: K(3, 3)],
                         8: [lambda: Q(3, 1)]},
                (3, 0): {2: [lambda: Q(3, 2)], 5: [lambda: Q(3, 3)]},
                (3, 1): {(2 * i + 1): [lambda oc=i: O(0, oc)]
                         for i in range(8)},
                (3, 2): {(2 * i + 1): [lambda oc=i: O(1, oc)]
                         for i in range(8)},
                (3, 3): {(2 * i + 1): [lambda oc=i: O(2, oc)]
                         for i in range(8)},
            }

            for hp in range(4):
                for qb in range(4):
                    emit_block(hp, qb, SCHED[(hp, qb)])

            # tail: last q-block's out-projection (yt copies ride the
            # now-idle Scalar engine)
            for oc in range(8):
                O(3, oc, tail=True)

    nc.compile()
    return nc


def _get_nc():
    if "nc" not in _cache:
        _cache["nc"] = _build_nc()
    return _cache["nc"]


def kernel(q, k, v, mask, Wq, bq, Wk, bk, Wv, bv, Wo, bo):
    from concourse.bass_utils import run_bass_kernel_spmd

    nc = _get_nc()

    in_maps = []
    for c in range(8):
        b, g = c // 2, c % 2
        gsl = slice(g * DL, (g + 1) * DL)
        in_maps.append({
            "xqT": np.ascontiguousarray(np.asarray(q[b], np.float32).T).astype(BF16),
            "xkT": np.ascontiguousarray(np.asarray(k[b], np.float32).T).astype(BF16),
            "xvT": np.ascontiguousarray(np.asarray(v[b], np.float32).T).astype(BF16),
            "wqT": np.ascontiguousarray(np.asarray(Wq, np.float32)[gsl, :].T).astype(BF16),
            "wkT": np.ascontiguousarray(np.asarray(Wk, np.float32)[gsl, :].T).astype(BF16),
            "wvT": np.ascontiguousarray(np.asarray(Wv, np.float32)[gsl, :].T).astype(BF16),
            "woT": np.ascontiguousarray(np.asarray(Wo, np.float32)[:, gsl].T).astype(BF16),
            "bq2": np.ascontiguousarray(np.asarray(bq, np.float32)[gsl].reshape(4, 128).T),
            "bk2": np.ascontiguousarray(np.asarray(bk, np.float32)[gsl].reshape(4, 128).T),
        })

    _cache["in_maps"] = in_maps
    res = run_bass_kernel_spmd(nc, in_maps, list(range(8)))
    _cache["last_results"] = res

    # v-bias folded here: reference adds bv per head dim before the out
    # projection, so its contribution is the constant vector Wo @ bv
    bias = np.asarray(bo, np.float32) + np.asarray(Wo, np.float32) @ np.asarray(bv, np.float32)
    out = np.empty((NB, S, D), np.float32)
    for b in range(NB):
        y0 = res.results[2 * b]["yT"].astype(np.float32)
        y1 = res.results[2 * b + 1]["yT"].astype(np.float32)
        out[b] = (y0 + y1).T + bias
    return out
